# revision 7
# baseline (speedup 1.0000x reference)
"""LocalAttention Trainium2 Bass kernel (folded weights, fp16 compute).

Problem: B=8, L=7936, C=256, WINDOW=31.  y = proj(attn(qkv(x))) with
window-local softmax (nW=256 windows of 31 tokens per batch element).

Sharding: data-parallel over batch; core b handles x[b] [7936, 256].

Algebraic folding (exact):
  scores*256 = x^T A16 x with A16 = (4Wq)^T (4Wk) = 16 Wq^T Wk applied as
    z = A16^T-contracted: z[c,q] = sum_cin A16'[cin,c] x[cin,q] with
    A16' = (4Wq)^T(4Wk) built so z = 16 Wk^T Wq x ... (verified vs reference)
  exp(psum/256); bq folds into a per-partition bias on z; the bk term is a
    per-query shift that softmax cancels exactly.
  y: Wu16 = 16 Wproj Wv; u = Wu16-applied + 16 bp; y = (sum_k e_k u_k)/(16 sum_k e_k)
  Sums use a 16-valued ones column so the 16s cancel in the reciprocal.

Per-core dataflow:
  - x quad [512 tok] f32 --SWDGE cast DMA--> fp16 staging -> PE transpose
    -> one PSUM bank -> DVE copy -> resident xT16 [c_lo, c_hi, t] fp16
  - z = A x: fp16 matmuls, 2 c-slices accumulated
  - scores per 124-token block: xT16 keys (lhsT) x z16 queries
  - exp on ACT (scale 1/256) -> attn fp16; window mask on GPSIMD
  - sums: 4 tiny PE matmuls into spare cols of the scores bank; DVE recip
  - u = Wu x: fp16, token-major, 124-aligned blocks
  - AV: y[t,c] = sum_k attn[k,t] u16[k,c]; y-scale: DVE tensor_scalar -> DMA

PSUM (8 banks): xt 1 (bufs=1) + z 2 (bufs=1) + s 2 (bufs=2, sums share bank)
  + uy 3 (shared u/y half-tile ring, bufs=3).
"""

import sys

for _p in ("/opt/trn_rl_repo",):
    if _p not in sys.path:
        sys.path.insert(0, _p)

import numpy as np

import concourse.bass as bass
import concourse.bacc as bacc_mod
import concourse.tile as tile
from concourse import mybir
from concourse.masks import make_identity

F32 = mybir.dt.float32
F16 = mybir.dt.float16
Identity = mybir.ActivationFunctionType.Identity
Exp = mybir.ActivationFunctionType.Exp

B, L, C = 8, 7936, 256
WS = 31
BLK = 4 * WS            # 124 tokens per attention block
GRP = 4 * BLK           # 496 tokens per group
N_GRP = L // GRP        # 16
QUAD = 512              # x staging granularity (tokens)
N_QUAD = (L + QUAD - 1) // QUAD   # 16 (last covers 256)
LP = L + 32             # padded xT16 width

_CACHE = {}


def _build_nc(has_bqkv, has_bproj):
    nc = bacc_mod.Bacc("TRN2", target_bir_lowering=False, debug=False, num_devices=8)
    x_d = nc.dram_tensor("x", [L, C], F32, kind="ExternalInput").ap()
    wqkv_d = nc.dram_tensor("wqkv", [3 * C, C], F32, kind="ExternalInput").ap()
    bqkv_d = nc.dram_tensor("bqkv", [3 * C], F32, kind="ExternalInput").ap()
    wproj_d = nc.dram_tensor("wproj", [C, C], F32, kind="ExternalInput").ap()
    bproj_d = nc.dram_tensor("bproj", [C], F32, kind="ExternalInput").ap()
    y_d = nc.dram_tensor("y", [L, C], F32, kind="ExternalOutput").ap()

    with tile.TileContext(nc) as tc:
        _emit(tc, x_d, wqkv_d, bqkv_d, wproj_d, bproj_d, y_d, has_bqkv, has_bproj)
    nc.compile()
    return nc


def _emit(tc, x_d, wqkv_d, bqkv_d, wproj_d, bproj_d, y_d, has_bqkv, has_bproj):
    nc = tc.nc
    from contextlib import ExitStack

    ctx = ExitStack()
    consts = ctx.enter_context(tc.tile_pool(name="consts", bufs=1))
    xt_pool = ctx.enter_context(tc.tile_pool(name="xt", bufs=1))
    xstage = ctx.enter_context(tc.tile_pool(name="xstage", bufs=3))
    z_pool = ctx.enter_context(tc.tile_pool(name="zp", bufs=3))
    attn_pool = ctx.enter_context(tc.tile_pool(name="ap", bufs=3))
    u_pool = ctx.enter_context(tc.tile_pool(name="up", bufs=3))
    y_pool = ctx.enter_context(tc.tile_pool(name="yp", bufs=3))
    small_pool = ctx.enter_context(tc.tile_pool(name="sp", bufs=3))

    # PSUM budget (8 banks): xt 1 + z 2 + s 2 + uy 3
    ps = ctx.enter_context(tc.tile_pool(name="ps", bufs=1, space="PSUM"))

    def ps_xt(name):
        return ps.tile([128, 4, 2, 128], F16, tag="xt", bufs=1, name=name)

    def ps_z(name):
        return ps.tile([128, 2, 512], F32, tag="z", bufs=1, name=name)

    def ps_s(name):
        return ps.tile([128, 512], F32, tag="s", bufs=2, name=name)

    def ps_uy(name):
        return ps.tile([128, 2, C], F32, tag="uy", bufs=3, name=name)

    # ---------------- constants / folded weights ----------------
    ident16 = consts.tile([128, 128], F16)
    make_identity(nc, ident16[:])

    # Wqkv [768,256] -> j-major [128, 6, 256] fp16 (x4); o 0-1=q, 2-3=k, 4-5=v
    w_raw = consts.tile([128, 6, C], F32)
    nc.sync.dma_start(w_raw[:], wqkv_d.rearrange("(o p) c -> p o c", p=128))

    # hoisted x staging (defined early so quads can overlap weights prep)
    xT16 = xt_pool.tile([128, 2, LP], F16)
    nc.gpsimd.memset(xT16[:, :, L:LP], 0.0)

    def emit_quad(q):
        t0 = q * QUAD
        ntt = min(4, (L - t0) // 128)
        x16 = xstage.tile([128, 4, C], F16, tag="x16", name=f"x16_{q}")
        nc.gpsimd.dma_start(
            x16[:, 0:ntt, :],
            x_d[t0:t0 + ntt * 128, :].rearrange("(a p) c -> p a c", p=128),
        )
        pt = ps.tile([128, 4, 2, 128], F16, tag="xt", bufs=1, name=f"xtp_{q}")
        for tt in range(ntt):
            for cs in range(2):
                nc.tensor.transpose(
                    pt[:, tt, cs, :], x16[:, tt, cs * 128:(cs + 1) * 128], ident16[:]
                )
        nc.vector.tensor_copy(
            xT16[:, :, t0:t0 + ntt * 128].rearrange("p c (t k) -> p c t k", t=ntt),
            pt[:, 0:ntt].rearrange("p t c k -> p c t k"),
        )

    for _q in range(3):
        emit_quad(_q)

    w16 = consts.tile([128, 6, C], F16)
    nc.scalar.activation(w16[:], w_raw[:], Identity, bias=0.0, scale=4.0)

    # Wproj [256,256] -> jo-major fp16 (x4), and its transpose [m_lo, m_hi, jo]
    wp_raw = consts.tile([128, 2, C], F32)
    nc.sync.dma_start(wp_raw[:], wproj_d.rearrange("(o p) c -> p o c", p=128))
    wp16 = consts.tile([128, 2, C], F16)
    nc.scalar.activation(wp16[:], wp_raw[:], Identity, bias=0.0, scale=4.0)

    wp16T = consts.tile([128, 2, C], F16)
    ptw = ps.tile([128, 4, 2, 128], F16, tag="xt", bufs=1, name="ptw")
    for o in range(2):          # jo tile
        for ms in range(2):     # m slice
            nc.tensor.transpose(
                ptw[:, o, ms, :], wp16[:, o, ms * 128:(ms + 1) * 128], ident16[:]
            )
    nc.vector.tensor_copy(
        wp16T[:].rearrange("p m (o k) -> p o m k", o=2),
        ptw[:, 0:2],
    )

    # A16 [cin_lo, cin_hi, cout] = (4Wq)^T (4Wk);  Wu16 [cin_lo, cin_hi, jo]
    A16 = consts.tile([128, 2, C], F16)
    Wu16 = consts.tile([128, 2, C], F16)
    for a in range(2):
        pa = ps_uy(f"wprep_a{a}")
        for js in range(2):
            nc.tensor.matmul(
                pa[:, 0, :], w16[:, js, a * 128:(a + 1) * 128], w16[:, 2 + js, :],
                start=(js == 0), stop=(js == 1),
            )
        for ms in range(2):
            nc.tensor.matmul(
                pa[:, 1, :], w16[:, 4 + ms, a * 128:(a + 1) * 128], wp16T[:, ms, :],
                start=(ms == 0), stop=(ms == 1),
            )
        nc.scalar.copy(A16[:, a, :], pa[:, 0, :])
        nc.scalar.copy(Wu16[:, a, :], pa[:, 1, :])

    # optional bias folds
    zb = None
    if has_bqkv:
        bq_sb = consts.tile([128, 6, 1], F32)
        nc.sync.dma_start(bq_sb[:], bqkv_d.rearrange("(o p) -> p o", p=128)[:, :, None])
        zb = consts.tile([128, 2], F32)
        for a in range(2):
            pb = ps_s(f"zb_{a}")
            for js in range(2):
                nc.tensor.matmul(
                    pb[:, 0:1], w_raw[:, 2 + js, a * 128:(a + 1) * 128],
                    bq_sb[:, js, :], start=(js == 0), stop=(js == 1),
                )
            nc.scalar.activation(zb[:, a:a + 1], pb[:, 0:1], Identity,
                                 bias=0.0, scale=16.0)
    ub_bc = None
    if has_bproj:
        bp_row = consts.tile([1, C], F32)
        nc.sync.dma_start(bp_row[:], bproj_d[None, :])
        bp16 = consts.tile([1, C], F16)
        nc.scalar.activation(bp16[:], bp_row[:], Identity, bias=0.0, scale=16.0)
        ones_row = consts.tile([1, BLK], F16)
        nc.vector.memset(ones_row[:], 1.0)
        pbb = ps_uy("ubprep")
        nc.tensor.matmul(pbb[0:BLK, 0, :], ones_row[:], bp16[:], start=True, stop=True)
        ub_bc = consts.tile([BLK, 2, C], F32)
        for b2 in range(2):
            nc.scalar.copy(ub_bc[:, b2, :], pbb[0:BLK, 0, :])

    # block-diag window mask [124, 496] fp16
    mask_np = np.zeros((BLK, GRP), dtype=np.float16)
    for j4 in range(4):
        for w in range(4):
            mask_np[w * WS:(w + 1) * WS, j4 * BLK + w * WS: j4 * BLK + (w + 1) * WS] = 1.0
    mask_d = nc.inline_tensor(mask_np, name="maskc")
    mask_sb = consts.tile([BLK, GRP], F16)
    nc.sync.dma_start(mask_sb[:], mask_d.ap())

    ones16 = consts.tile([BLK, 1], F16)
    nc.vector.memset(ones16[:], 16.0)

    # ---------------- software-pipelined loop (skew 3) ----------------
    # iter i: quads(i+1) | S0: z(i) | S1: scores/exp/mask(i-1)
    #         | S2: sums/recip/u(i-2) | S3: AV/y-scale/DMA(i-3)
    state = {}
    quads_done = 3

    def s0(g):
        t0 = g * GRP
        pz = ps_z(f"pz_{g}")
        for jt in range(2):
            for cs in range(2):
                nc.tensor.matmul(
                    pz[:, jt, 0:GRP], A16[:, cs, jt * 128:(jt + 1) * 128],
                    xT16[:, cs, t0:t0 + GRP], start=(cs == 0), stop=(cs == 1),
                )
        z16 = z_pool.tile([128, 2, GRP], F16, tag="z16", name=f"z16_{g}")
        if zb is None:
            nc.scalar.activation(z16[:], pz[:, :, 0:GRP], Identity, bias=0.0, scale=1.0)
        else:
            for jt in range(2):
                nc.scalar.activation(z16[:, jt, :], pz[:, jt, 0:GRP], Identity,
                                     bias=zb[:, jt:jt + 1], scale=1.0)
        state[("z16", g)] = z16

    def s1(g):
        t0 = g * GRP
        z16 = state.pop(("z16", g))
        psc = ps_s(f"psc_{g}")
        for b4 in range(4):
            for cs in range(2):
                nc.tensor.matmul(
                    psc[:, b4 * BLK:(b4 + 1) * BLK],
                    xT16[:, cs, t0 + b4 * BLK: t0 + b4 * BLK + 128],
                    z16[:, cs, b4 * BLK:(b4 + 1) * BLK],
                    start=(cs == 0), stop=(cs == 1),
                )
        attn = attn_pool.tile([BLK, GRP], F16, tag="at", name=f"at_{g}")
        nc.scalar.activation(attn[:], psc[0:BLK, 0:GRP], Exp, scale=1.0 / 256.0)
        nc.gpsimd.tensor_mul(attn[:], attn[:], mask_sb[:])
        state[("psc", g)] = psc
        state[("attn", g)] = attn

    def s2(g):
        t0 = g * GRP
        psc = state.pop(("psc", g))
        attn = state[("attn", g)]
        for b4 in range(4):
            nc.tensor.matmul(
                psc[0:BLK, GRP + b4:GRP + b4 + 1],
                attn[:, b4 * BLK:(b4 + 1) * BLK], ones16[:],
                start=True, stop=True,
            )
        recip = small_pool.tile([BLK, 4], F32, tag="rc", name=f"rc_{g}")
        nc.vector.reciprocal(recip[:], psc[0:BLK, GRP:GRP + 4])
        u16 = u_pool.tile([BLK, 4, C], F16, tag="u16", name=f"u16_{g}")
        for h in range(2):
            pu = ps_uy(f"pu_{g}_{h}")
            for b2 in range(2):
                b4 = 2 * h + b2
                for cs in range(2):
                    nc.tensor.matmul(
                        pu[0:BLK, b2, :],
                        xT16[:, cs, t0 + b4 * BLK: t0 + b4 * BLK + BLK],
                        Wu16[:, cs, :], start=(cs == 0), stop=(cs == 1),
                    )
            if ub_bc is not None:
                nc.vector.tensor_add(u16[:, 2 * h:2 * h + 2, :], pu[0:BLK, :, :],
                                     ub_bc[:])
            elif h == 1 and g % 2 == 1:
                nc.vector.tensor_copy(u16[:, 2 * h:2 * h + 2, :], pu[0:BLK, :, :])
            else:
                nc.scalar.copy(u16[:, 2 * h:2 * h + 2, :], pu[0:BLK, :, :])
        state[("recip", g)] = recip
        state[("u16", g)] = u16

    def s3(g):
        t0 = g * GRP
        attn = state.pop(("attn", g))
        recip = state.pop(("recip", g))
        u16 = state.pop(("u16", g))
        y_sb = y_pool.tile([BLK, 4, C], F32, tag="y", name=f"y_{g}")
        for h in range(2):
            py = ps_uy(f"py_{g}_{h}")
            for b2 in range(2):
                b4 = 2 * h + b2
                nc.tensor.matmul(
                    py[0:BLK, b2, :],
                    attn[:, b4 * BLK:(b4 + 1) * BLK], u16[:, b4, :],
                    start=True, stop=True,
                )
            for b2 in range(2):
                b4 = 2 * h + b2
                nc.vector.tensor_scalar(
                    y_sb[:, b4, :], py[0:BLK, b2, :], recip[:, b4:b4 + 1], None,
                    mybir.AluOpType.mult,
                )
        nc.sync.dma_start(
            y_d[t0:t0 + GRP, :].rearrange("(a p) c -> p a c", p=BLK), y_sb[:]
        )

    for i in range(N_GRP + 3):
        if i - 3 >= 0:
            s3(i - 3)          # AV first: unblocks DVE y-scale early
        if i < N_GRP:
            # prefetch quads for group i+1 (group i's quads already resident)
            gq = min(i + 1, N_GRP - 1)
            need = min(N_QUAD, (gq * GRP + GRP + QUAD - 1) // QUAD)
            while quads_done < need:
                emit_quad(quads_done)
                quads_done += 1
            s0(i)
        if 0 <= i - 1 < N_GRP:
            s1(i - 1)
        if 0 <= i - 2 < N_GRP:
            s2(i - 2)

    ctx.close()


def kernel(x, Wqkv, bqkv, Wproj, bproj):
    from concourse.bass_utils import run_bass_kernel_spmd

    x = np.ascontiguousarray(np.asarray(x, dtype=np.float32))
    wqkv = np.ascontiguousarray(np.asarray(Wqkv, dtype=np.float32))
    bq = np.ascontiguousarray(np.asarray(bqkv, dtype=np.float32))
    wproj = np.ascontiguousarray(np.asarray(Wproj, dtype=np.float32))
    bp = np.ascontiguousarray(np.asarray(bproj, dtype=np.float32))

    key = (bool(np.any(bq)), bool(np.any(bp)))
    if _CACHE.get("key") != key:
        _CACHE["nc"] = _build_nc(*key)
        _CACHE["key"] = key
    nc = _CACHE["nc"]

    in_maps = [
        {"x": x[b], "wqkv": wqkv, "bqkv": bq, "wproj": wproj, "bproj": bp}
        for b in range(B)
    ]
    res = run_bass_kernel_spmd(nc, in_maps, core_ids=list(range(B)))
    return np.stack([r["y"] for r in res.results], axis=0)


# revision 9
# speedup vs baseline: 1.0135x; 1.0135x over previous
"""LocalAttention Trainium2 Bass kernel (folded weights, fp16 compute).

Problem: B=8, L=7936, C=256, WINDOW=31.  y = proj(attn(qkv(x))) with
window-local softmax (nW=256 windows of 31 tokens per batch element).

Sharding: data-parallel over batch; core b handles x[b] [7936, 256].

Algebraic folding (exact):
  scores*256 = x^T A16 x with A16 = (4Wq)^T (4Wk) = 16 Wq^T Wk applied as
    z = A16^T-contracted: z[c,q] = sum_cin A16'[cin,c] x[cin,q] with
    A16' = (4Wq)^T(4Wk) built so z = 16 Wk^T Wq x ... (verified vs reference)
  exp(psum/256); bq folds into a per-partition bias on z; the bk term is a
    per-query shift that softmax cancels exactly.
  y: Wu16 = 16 Wproj Wv; u = Wu16-applied + 16 bp; y = (sum_k e_k u_k)/(16 sum_k e_k)
  Sums use a 16-valued ones column so the 16s cancel in the reciprocal.

Per-core dataflow:
  - x quad [512 tok] f32 --SWDGE cast DMA--> fp16 staging -> PE transpose
    -> one PSUM bank -> DVE copy -> resident xT16 [c_lo, c_hi, t] fp16
  - z = A x: fp16 matmuls, 2 c-slices accumulated
  - scores per 124-token block: xT16 keys (lhsT) x z16 queries
  - exp on ACT (scale 1/256) -> attn fp16; window mask on GPSIMD
  - sums: 4 tiny PE matmuls into spare cols of the scores bank; DVE recip
  - u = Wu x: fp16, token-major, 124-aligned blocks
  - AV: y[t,c] = sum_k attn[k,t] u16[k,c]; y-scale: DVE tensor_scalar -> DMA

PSUM (8 banks): xt 1 (bufs=1) + z 2 (bufs=1) + s 2 (bufs=2, sums share bank)
  + uy 3 (shared u/y half-tile ring, bufs=3).
"""

import sys

for _p in ("/opt/trn_rl_repo",):
    if _p not in sys.path:
        sys.path.insert(0, _p)

import numpy as np

import concourse.bass as bass
import concourse.bacc as bacc_mod
import concourse.tile as tile
from concourse import mybir
from concourse.masks import make_identity

F32 = mybir.dt.float32
F16 = mybir.dt.float16
F8 = mybir.dt.float8e4
DR = mybir.MatmulPerfMode.DoubleRow
Identity = mybir.ActivationFunctionType.Identity
Exp = mybir.ActivationFunctionType.Exp

B, L, C = 8, 7936, 256
WS = 31
BLK = 4 * WS            # 124 tokens per attention block
GRP = 4 * BLK           # 496 tokens per group
N_GRP = L // GRP        # 16
QUAD = 512              # x staging granularity (tokens)
N_QUAD = (L + QUAD - 1) // QUAD   # 16 (last covers 256)
LP = L + 32             # padded xT16 width

_CACHE = {}


def _build_nc(has_bqkv, has_bproj):
    nc = bacc_mod.Bacc("TRN2", target_bir_lowering=False, debug=False, num_devices=8)
    x_d = nc.dram_tensor("x", [L, C], F32, kind="ExternalInput").ap()
    wqkv_d = nc.dram_tensor("wqkv", [3 * C, C], F32, kind="ExternalInput").ap()
    bqkv_d = nc.dram_tensor("bqkv", [3 * C], F32, kind="ExternalInput").ap()
    wproj_d = nc.dram_tensor("wproj", [C, C], F32, kind="ExternalInput").ap()
    bproj_d = nc.dram_tensor("bproj", [C], F32, kind="ExternalInput").ap()
    y_d = nc.dram_tensor("y", [L, C], F32, kind="ExternalOutput").ap()

    with tile.TileContext(nc) as tc:
        _emit(tc, x_d, wqkv_d, bqkv_d, wproj_d, bproj_d, y_d, has_bqkv, has_bproj)
    nc.compile()
    return nc


def _emit(tc, x_d, wqkv_d, bqkv_d, wproj_d, bproj_d, y_d, has_bqkv, has_bproj):
    nc = tc.nc
    from contextlib import ExitStack

    ctx = ExitStack()
    consts = ctx.enter_context(tc.tile_pool(name="consts", bufs=1))
    xt_pool = ctx.enter_context(tc.tile_pool(name="xt", bufs=1))
    xstage = ctx.enter_context(tc.tile_pool(name="xstage", bufs=3))
    z_pool = ctx.enter_context(tc.tile_pool(name="zp", bufs=3))
    attn_pool = ctx.enter_context(tc.tile_pool(name="ap", bufs=3))
    u_pool = ctx.enter_context(tc.tile_pool(name="up", bufs=3))
    y_pool = ctx.enter_context(tc.tile_pool(name="yp", bufs=3))
    small_pool = ctx.enter_context(tc.tile_pool(name="sp", bufs=3))

    # PSUM budget (8 banks): xt 1 + z 2 + s 2 + uy 3
    ps = ctx.enter_context(tc.tile_pool(name="ps", bufs=1, space="PSUM"))

    def ps_xt(name):
        return ps.tile([128, 4, 2, 128], F16, tag="xt", bufs=1, name=name)

    def ps_z(name):
        return ps.tile([128, 2, 512], F32, tag="z", bufs=1, name=name)

    def ps_s(name):
        return ps.tile([128, 512], F32, tag="s", bufs=2, name=name)

    def ps_uy(name):
        return ps.tile([128, 2, C], F32, tag="uy", bufs=3, name=name)

    # ---------------- constants / folded weights ----------------
    # Wqkv [768,256] -> j-major [128, 6, 256] fp16 (x4); o 0-1=q, 2-3=k, 4-5=v
    w_raw = consts.tile([128, 6, C], F32)
    nc.sync.dma_start(w_raw[:], wqkv_d.rearrange("(o p) c -> p o c", p=128))

    # hoisted x staging: start the first x cast-DMAs before anything else
    xT16 = xt_pool.tile([128, 2, LP], F16)
    _xstage_tiles = {}

    def start_quad_dma(q):
        t0 = q * QUAD
        ntt = min(4, (L - t0) // 128)
        x16 = xstage.tile([128, 4, C], F16, tag="x16", name=f"x16_{q}")
        nc.gpsimd.dma_start(
            x16[:, 0:ntt, :],
            x_d[t0:t0 + ntt * 128, :].rearrange("(a p) c -> p a c", p=128),
        )
        _xstage_tiles[q] = (x16, ntt)

    def finish_quad(q):
        t0 = q * QUAD
        x16, ntt = _xstage_tiles.pop(q)
        pt = ps.tile([128, 4, 2, 128], F16, tag="xt", bufs=1, name=f"xtp_{q}")
        for tt in range(ntt):
            for cs in range(2):
                nc.tensor.transpose(
                    pt[:, tt, cs, :], x16[:, tt, cs * 128:(cs + 1) * 128], ident16[:]
                )
        nc.vector.tensor_copy(
            xT16[:, :, t0:t0 + ntt * 128].rearrange("p c (t k) -> p c t k", t=ntt),
            pt[:, 0:ntt].rearrange("p t c k -> p c t k"),
        )

    def emit_quad(q):
        start_quad_dma(q)
        finish_quad(q)

    for _q in range(3):
        start_quad_dma(_q)

    ident16 = consts.tile([128, 128], F16)
    make_identity(nc, ident16[:])

    w16 = consts.tile([128, 6, C], F16)
    nc.scalar.activation(w16[:], w_raw[:], Identity, bias=0.0, scale=4.0)

    # Wproj [256,256] -> jo-major fp16 (x4), and its transpose [m_lo, m_hi, jo]
    wp_raw = consts.tile([128, 2, C], F32)
    nc.sync.dma_start(wp_raw[:], wproj_d.rearrange("(o p) c -> p o c", p=128))
    wp16 = consts.tile([128, 2, C], F16)
    nc.scalar.activation(wp16[:], wp_raw[:], Identity, bias=0.0, scale=4.0)

    wp16T = consts.tile([128, 2, C], F16)
    ptw = ps.tile([128, 4, 2, 128], F16, tag="xt", bufs=1, name="ptw")
    for o in range(2):          # jo tile
        for ms in range(2):     # m slice
            nc.tensor.transpose(
                ptw[:, o, ms, :], wp16[:, o, ms * 128:(ms + 1) * 128], ident16[:]
            )
    nc.vector.tensor_copy(
        wp16T[:].rearrange("p m (o k) -> p o m k", o=2),
        ptw[:, 0:2],
    )

    # A16 [cin_lo, cin_hi, cout] = (4Wq)^T (4Wk);  Wu16 [cin_lo, cin_hi, jo]
    A16 = consts.tile([128, 2, C], F16)
    Wu16 = consts.tile([128, 2, C], F16)
    for a in range(2):
        pa = ps_uy(f"wprep_a{a}")
        for js in range(2):
            nc.tensor.matmul(
                pa[:, 0, :], w16[:, js, a * 128:(a + 1) * 128], w16[:, 2 + js, :],
                start=(js == 0), stop=(js == 1),
            )
        for ms in range(2):
            nc.tensor.matmul(
                pa[:, 1, :], w16[:, 4 + ms, a * 128:(a + 1) * 128], wp16T[:, ms, :],
                start=(ms == 0), stop=(ms == 1),
            )
        nc.scalar.copy(A16[:, a, :], pa[:, 0, :])
        nc.scalar.copy(Wu16[:, a, :], pa[:, 1, :])

    for _q in range(3):
        finish_quad(_q)

    # optional bias folds
    zb = None
    if has_bqkv:
        bq_sb = consts.tile([128, 6, 1], F32)
        nc.sync.dma_start(bq_sb[:], bqkv_d.rearrange("(o p) -> p o", p=128)[:, :, None])
        zb = consts.tile([128, 2], F32)
        for a in range(2):
            pb = ps_s(f"zb_{a}")
            for js in range(2):
                nc.tensor.matmul(
                    pb[:, 0:1], w_raw[:, 2 + js, a * 128:(a + 1) * 128],
                    bq_sb[:, js, :], start=(js == 0), stop=(js == 1),
                )
            nc.scalar.activation(zb[:, a:a + 1], pb[:, 0:1], Identity,
                                 bias=0.0, scale=16.0)
    ub_bc = None
    if has_bproj:
        bp_row = consts.tile([1, C], F32)
        nc.sync.dma_start(bp_row[:], bproj_d[None, :])
        bp16 = consts.tile([1, C], F16)
        nc.scalar.activation(bp16[:], bp_row[:], Identity, bias=0.0, scale=16.0)
        ones_row = consts.tile([1, BLK], F16)
        nc.vector.memset(ones_row[:], 1.0)
        pbb = ps_uy("ubprep")
        nc.tensor.matmul(pbb[0:BLK, 0, :], ones_row[:], bp16[:], start=True, stop=True)
        ub_bc = consts.tile([BLK, 2, C], F32)
        for b2 in range(2):
            nc.scalar.copy(ub_bc[:, b2, :], pbb[0:BLK, 0, :])

    # block-diag window mask [124, 496] fp16
    mask_np = np.zeros((BLK, GRP), dtype=np.float16)
    for j4 in range(4):
        for w in range(4):
            mask_np[w * WS:(w + 1) * WS, j4 * BLK + w * WS: j4 * BLK + (w + 1) * WS] = 1.0
    mask_d = nc.inline_tensor(mask_np, name="maskc")
    mask_sb = consts.tile([BLK, GRP], F16)
    nc.sync.dma_start(mask_sb[:], mask_d.ap())

    ones16 = consts.tile([BLK, 1], F16)
    nc.vector.memset(ones16[:], 16.0)

    # ---------------- software-pipelined loop (skew 3) ----------------
    # iter i: quads(i+1) | S0: z(i) | S1: scores/exp/mask(i-1)
    #         | S2: sums/recip/u(i-2) | S3: AV/y-scale/DMA(i-3)
    state = {}
    quads_done = 3

    def s0(g):
        t0 = g * GRP
        pz = ps_z(f"pz_{g}")
        for jt in range(2):
            for cs in range(2):
                nc.tensor.matmul(
                    pz[:, jt, 0:GRP], A16[:, cs, jt * 128:(jt + 1) * 128],
                    xT16[:, cs, t0:t0 + GRP], start=(cs == 0), stop=(cs == 1),
                )
        z16 = z_pool.tile([128, 2, GRP], F16, tag="z16", name=f"z16_{g}")
        if zb is None:
            nc.scalar.activation(z16[:], pz[:, :, 0:GRP], Identity, bias=0.0, scale=1.0)
        else:
            for jt in range(2):
                nc.scalar.activation(z16[:, jt, :], pz[:, jt, 0:GRP], Identity,
                                     bias=zb[:, jt:jt + 1], scale=1.0)
        state[("z16", g)] = z16

    def s1(g):
        t0 = g * GRP
        z16 = state.pop(("z16", g))
        psc = ps_s(f"psc_{g}")
        for b4 in range(4):
            for cs in range(2):
                nc.tensor.matmul(
                    psc[:, b4 * BLK:(b4 + 1) * BLK],
                    xT16[:, cs, t0 + b4 * BLK: t0 + b4 * BLK + 128],
                    z16[:, cs, b4 * BLK:(b4 + 1) * BLK],
                    start=(cs == 0), stop=(cs == 1),
                )
        attn = attn_pool.tile([BLK, GRP], F16, tag="at", name=f"at_{g}")
        nc.scalar.activation(attn[:], psc[0:BLK, 0:GRP], Exp, scale=1.0 / 256.0)
        nc.gpsimd.tensor_mul(attn[:], attn[:], mask_sb[:])
        state[("psc", g)] = psc
        state[("attn", g)] = attn

    def s2(g):
        t0 = g * GRP
        psc = state.pop(("psc", g))
        attn = state[("attn", g)]
        for b4 in range(4):
            nc.tensor.matmul(
                psc[0:BLK, GRP + b4:GRP + b4 + 1],
                attn[:, b4 * BLK:(b4 + 1) * BLK], ones16[:],
                start=True, stop=True,
            )
        recip = small_pool.tile([BLK, 4], F32, tag="rc", name=f"rc_{g}")
        nc.vector.reciprocal(recip[:], psc[0:BLK, GRP:GRP + 4])
        u16 = u_pool.tile([BLK, 4, C], F16, tag="u16", name=f"u16_{g}")
        for h in range(2):
            pu = ps_uy(f"pu_{g}_{h}")
            for b2 in range(2):
                b4 = 2 * h + b2
                for cs in range(2):
                    nc.tensor.matmul(
                        pu[0:BLK, b2, :],
                        xT16[:, cs, t0 + b4 * BLK: t0 + b4 * BLK + BLK],
                        Wu16[:, cs, :], start=(cs == 0), stop=(cs == 1),
                    )
            if ub_bc is not None:
                nc.vector.tensor_add(u16[:, 2 * h:2 * h + 2, :], pu[0:BLK, :, :],
                                     ub_bc[:])
            elif h == 1 and g % 2 == 1:
                nc.vector.tensor_copy(u16[:, 2 * h:2 * h + 2, :], pu[0:BLK, :, :])
            else:
                nc.scalar.copy(u16[:, 2 * h:2 * h + 2, :], pu[0:BLK, :, :])
        state[("recip", g)] = recip
        state[("u16", g)] = u16

    def s3(g):
        t0 = g * GRP
        attn = state.pop(("attn", g))
        recip = state.pop(("recip", g))
        u16 = state.pop(("u16", g))
        y_sb = y_pool.tile([BLK, 4, C], F32, tag="y", name=f"y_{g}")
        for h in range(2):
            py = ps_uy(f"py_{g}_{h}")
            for b2 in range(2):
                b4 = 2 * h + b2
                nc.tensor.matmul(
                    py[0:BLK, b2, :],
                    attn[:, b4 * BLK:(b4 + 1) * BLK], u16[:, b4, :],
                    start=True, stop=True,
                )
            for b2 in range(2):
                b4 = 2 * h + b2
                nc.vector.tensor_scalar(
                    y_sb[:, b4, :], py[0:BLK, b2, :], recip[:, b4:b4 + 1], None,
                    mybir.AluOpType.mult,
                )
        nc.sync.dma_start(
            y_d[t0:t0 + GRP, :].rearrange("(a p) c -> p a c", p=BLK), y_sb[:]
        )

    for i in range(N_GRP + 3):
        if i - 3 >= 0:
            s3(i - 3)          # AV first: unblocks DVE y-scale early
        if i < N_GRP:
            # prefetch quads for group i+1 (group i's quads already resident)
            gq = min(i + 1, N_GRP - 1)
            need = min(N_QUAD, (gq * GRP + GRP + QUAD - 1) // QUAD)
            while quads_done < need:
                emit_quad(quads_done)
                quads_done += 1
            s0(i)
        if 0 <= i - 1 < N_GRP:
            s1(i - 1)
        if 0 <= i - 2 < N_GRP:
            s2(i - 2)

    ctx.close()


def kernel(x, Wqkv, bqkv, Wproj, bproj):
    from concourse.bass_utils import run_bass_kernel_spmd

    x = np.ascontiguousarray(np.asarray(x, dtype=np.float32))
    wqkv = np.ascontiguousarray(np.asarray(Wqkv, dtype=np.float32))
    bq = np.ascontiguousarray(np.asarray(bqkv, dtype=np.float32))
    wproj = np.ascontiguousarray(np.asarray(Wproj, dtype=np.float32))
    bp = np.ascontiguousarray(np.asarray(bproj, dtype=np.float32))

    key = (bool(np.any(bq)), bool(np.any(bp)))
    if _CACHE.get("key") != key:
        _CACHE["nc"] = _build_nc(*key)
        _CACHE["key"] = key
    nc = _CACHE["nc"]

    in_maps = [
        {"x": x[b], "wqkv": wqkv, "bqkv": bq, "wproj": wproj, "bproj": bp}
        for b in range(B)
    ]
    res = run_bass_kernel_spmd(nc, in_maps, core_ids=list(range(B)))
    return np.stack([r["y"] for r in res.results], axis=0)


# revision 10
# speedup vs baseline: 1.0697x; 1.0555x over previous
"""LocalAttention Trainium2 Bass kernel (folded weights, fp16 compute).

Problem: B=8, L=7936, C=256, WINDOW=31.  y = proj(attn(qkv(x))) with
window-local softmax (nW=256 windows of 31 tokens per batch element).

Sharding: data-parallel over batch; core b handles x[b] [7936, 256].

Algebraic folding (exact):
  scores*256 = x^T A16 x with A16 = (4Wq)^T (4Wk) = 16 Wq^T Wk applied as
    z = A16^T-contracted: z[c,q] = sum_cin A16'[cin,c] x[cin,q] with
    A16' = (4Wq)^T(4Wk) built so z = 16 Wk^T Wq x ... (verified vs reference)
  exp(psum/256); bq folds into a per-partition bias on z; the bk term is a
    per-query shift that softmax cancels exactly.
  y: Wu16 = 16 Wproj Wv; u = Wu16-applied + 16 bp; y = (sum_k e_k u_k)/(16 sum_k e_k)
  Sums use a 16-valued ones column so the 16s cancel in the reciprocal.

Per-core dataflow:
  - x quad [512 tok] f32 --SWDGE cast DMA--> fp16 staging -> PE transpose
    -> one PSUM bank -> DVE copy -> resident xT16 [c_lo, c_hi, t] fp16
  - z = A x: fp16 matmuls, 2 c-slices accumulated
  - scores per 124-token block: xT16 keys (lhsT) x z16 queries
  - exp on ACT (scale 1/256) -> attn fp16; window mask on GPSIMD
  - sums: 4 tiny PE matmuls into spare cols of the scores bank; DVE recip
  - u = Wu x: fp16, token-major, 124-aligned blocks
  - AV: y[t,c] = sum_k attn[k,t] u16[k,c]; y-scale: DVE tensor_scalar -> DMA

PSUM (8 banks): xt 1 (bufs=1) + z 2 (bufs=1) + s 2 (bufs=2, sums share bank)
  + uy 3 (shared u/y half-tile ring, bufs=3).
"""

import sys

for _p in ("/opt/trn_rl_repo",):
    if _p not in sys.path:
        sys.path.insert(0, _p)

import numpy as np

import concourse.bass as bass
import concourse.bacc as bacc_mod
import concourse.tile as tile
from concourse import mybir
from concourse.masks import make_identity

F32 = mybir.dt.float32
F16 = mybir.dt.float16
F8 = mybir.dt.float8e4
DR = mybir.MatmulPerfMode.DoubleRow
Identity = mybir.ActivationFunctionType.Identity
Exp = mybir.ActivationFunctionType.Exp

B, L, C = 8, 7936, 256
WS = 31
BLK = 4 * WS            # 124 tokens per attention block
GRP = 4 * BLK           # 496 tokens per group
N_GRP = L // GRP        # 16
QUAD = 512              # x staging granularity (tokens)
N_QUAD = (L + QUAD - 1) // QUAD   # 16 (last covers 256)
LP = L + 32             # padded xT16 width

_CACHE = {}


def _build_nc(has_bqkv, has_bproj):
    nc = bacc_mod.Bacc("TRN2", target_bir_lowering=False, debug=False, num_devices=8)
    x_d = nc.dram_tensor("x", [L, C], F32, kind="ExternalInput").ap()
    wqkv_d = nc.dram_tensor("wqkv", [3 * C, C], F32, kind="ExternalInput").ap()
    bqkv_d = nc.dram_tensor("bqkv", [3 * C], F32, kind="ExternalInput").ap()
    wproj_d = nc.dram_tensor("wproj", [C, C], F32, kind="ExternalInput").ap()
    bproj_d = nc.dram_tensor("bproj", [C], F32, kind="ExternalInput").ap()
    y_d = nc.dram_tensor("y", [L, C], F32, kind="ExternalOutput").ap()

    with tile.TileContext(nc) as tc:
        _emit(tc, x_d, wqkv_d, bqkv_d, wproj_d, bproj_d, y_d, has_bqkv, has_bproj)
    nc.compile()
    return nc


def _emit(tc, x_d, wqkv_d, bqkv_d, wproj_d, bproj_d, y_d, has_bqkv, has_bproj):
    nc = tc.nc
    from contextlib import ExitStack

    ctx = ExitStack()
    consts = ctx.enter_context(tc.tile_pool(name="consts", bufs=1))
    xt_pool = ctx.enter_context(tc.tile_pool(name="xt", bufs=1))
    xstage = ctx.enter_context(tc.tile_pool(name="xstage", bufs=3))
    z_pool = ctx.enter_context(tc.tile_pool(name="zp", bufs=3))
    attn_pool = ctx.enter_context(tc.tile_pool(name="ap", bufs=3))
    u_pool = ctx.enter_context(tc.tile_pool(name="up", bufs=3))
    y_pool = ctx.enter_context(tc.tile_pool(name="yp", bufs=3))
    small_pool = ctx.enter_context(tc.tile_pool(name="sp", bufs=3))

    # PSUM budget (8 banks): xt 1 + z 2 + s 2 + uy 3
    ps = ctx.enter_context(tc.tile_pool(name="ps", bufs=1, space="PSUM"))

    def ps_xt(name):
        return ps.tile([128, 4, 2, 128], F16, tag="xt", bufs=1, name=name)

    def ps_z(name):
        return ps.tile([128, 2, 512], F32, tag="z", bufs=1, name=name)

    def ps_s(name):
        return ps.tile([128, 512], F32, tag="s", bufs=2, name=name)

    def ps_uy(name):
        return ps.tile([128, 2, C], F32, tag="uy", bufs=3, name=name)

    # ---------------- constants / folded weights ----------------
    ident16 = consts.tile([128, 128], F16)
    make_identity(nc, ident16[:])

    # x staging: chunked SWDGE cast-DMAs (f32 -> fp16). First two chunks are
    # single quads for a fast pipeline start; later chunks amortize the
    # descriptor-generation cost on the Pool engine.
    xT16 = xt_pool.tile([128, 2, LP], F16)
    CHUNKS = [(0, 1), (1, 2), (2, 6), (6, 10), (10, 14), (14, 16)]
    _chunk_tiles = {}
    _quad_chunk = {}
    for ci, (q0, q1) in enumerate(CHUNKS):
        for q in range(q0, q1):
            _quad_chunk[q] = ci

    def start_chunk_dma(ci):
        q0, q1 = CHUNKS[ci]
        t0 = q0 * QUAD
        ntt = (min(q1 * QUAD, L) - t0) // 128
        xc = xstage.tile([128, 16, C], F16, tag="x16", name=f"x16_{ci}")
        nc.gpsimd.dma_start(
            xc[:, 0:ntt, :],
            x_d[t0:t0 + ntt * 128, :].rearrange("(a p) c -> p a c", p=128),
        )
        _chunk_tiles[ci] = xc

    def finish_quad(q):
        t0 = q * QUAD
        ntt = min(4, (L - t0) // 128)
        ci = _quad_chunk[q]
        xc = _chunk_tiles[ci]
        o = (q - CHUNKS[ci][0]) * 4
        pt = ps.tile([128, 4, 2, 128], F16, tag="xt", bufs=1, name=f"xtp_{q}")
        for tt in range(ntt):
            for cs in range(2):
                nc.tensor.transpose(
                    pt[:, tt, cs, :], xc[:, o + tt, cs * 128:(cs + 1) * 128],
                    ident16[:],
                )
        nc.vector.tensor_copy(
            xT16[:, :, t0:t0 + ntt * 128].rearrange("p c (t k) -> p c t k", t=ntt),
            pt[:, 0:ntt].rearrange("p t c k -> p c t k"),
        )

    _chunks_started = [0]

    def ensure_chunk(q):
        ci = _quad_chunk[q]
        while _chunks_started[0] <= ci:
            start_chunk_dma(_chunks_started[0])
            _chunks_started[0] += 1

    def emit_quad(q):
        ensure_chunk(min(q + 2, N_QUAD - 1))  # stay 2 quads ahead on DMAs
        finish_quad(q)

    start_chunk_dma(0)
    _chunks_started[0] = 1

    # Wqkv [768,256] -> j-major [128, 6, 256] fp16 (x4); o 0-1=q, 2-3=k, 4-5=v
    w_raw = consts.tile([128, 6, C], F32)
    nc.sync.dma_start(w_raw[:], wqkv_d.rearrange("(o p) c -> p o c", p=128))
    ensure_chunk(2)

    wp_raw = consts.tile([128, 2, C], F32)
    nc.sync.dma_start(wp_raw[:], wproj_d.rearrange("(o p) c -> p o c", p=128))

    w16 = consts.tile([128, 6, C], F16)
    nc.scalar.activation(w16[:], w_raw[:], Identity, bias=0.0, scale=4.0)

    # A16 [cin_lo, cin_hi, cout] = (4Wq)^T (4Wk)  -- needs only w16
    A16 = consts.tile([128, 2, C], F16)
    Wu16 = consts.tile([128, 2, C], F16)
    for a in range(2):
        pa = ps_uy(f"wprep_a{a}")
        for js in range(2):
            nc.tensor.matmul(
                pa[:, 0, :], w16[:, js, a * 128:(a + 1) * 128], w16[:, 2 + js, :],
                start=(js == 0), stop=(js == 1),
            )
        nc.scalar.copy(A16[:, a, :], pa[:, 0, :])

    finish_quad(0)

    # Wproj -> jo-major fp16 (x4), transpose, then Wu16 = (4Wproj)(4Wv)
    wp16 = consts.tile([128, 2, C], F16)
    nc.scalar.activation(wp16[:], wp_raw[:], Identity, bias=0.0, scale=4.0)
    wp16T = consts.tile([128, 2, C], F16)
    ptw = ps.tile([128, 4, 2, 128], F16, tag="xt", bufs=1, name="ptw")
    for o in range(2):          # jo tile
        for ms in range(2):     # m slice
            nc.tensor.transpose(
                ptw[:, o, ms, :], wp16[:, o, ms * 128:(ms + 1) * 128], ident16[:]
            )
    nc.vector.tensor_copy(
        wp16T[:].rearrange("p m (o k) -> p o m k", o=2),
        ptw[:, 0:2],
    )
    for a in range(2):
        pw = ps_uy(f"wprep_u{a}")
        for ms in range(2):
            nc.tensor.matmul(
                pw[:, 0, :], w16[:, 4 + ms, a * 128:(a + 1) * 128], wp16T[:, ms, :],
                start=(ms == 0), stop=(ms == 1),
            )
        nc.scalar.copy(Wu16[:, a, :], pw[:, 0, :])

    for _q in range(1, 3):
        finish_quad(_q)

    # optional bias folds
    zb = None
    if has_bqkv:
        bq_sb = consts.tile([128, 6, 1], F32)
        nc.sync.dma_start(bq_sb[:], bqkv_d.rearrange("(o p) -> p o", p=128)[:, :, None])
        zb = consts.tile([128, 2], F32)
        for a in range(2):
            pb = ps_s(f"zb_{a}")
            for js in range(2):
                nc.tensor.matmul(
                    pb[:, 0:1], w_raw[:, 2 + js, a * 128:(a + 1) * 128],
                    bq_sb[:, js, :], start=(js == 0), stop=(js == 1),
                )
            nc.scalar.activation(zb[:, a:a + 1], pb[:, 0:1], Identity,
                                 bias=0.0, scale=16.0)
    ub_bc = None
    if has_bproj:
        bp_row = consts.tile([1, C], F32)
        nc.sync.dma_start(bp_row[:], bproj_d[None, :])
        bp16 = consts.tile([1, C], F16)
        nc.scalar.activation(bp16[:], bp_row[:], Identity, bias=0.0, scale=16.0)
        ones_row = consts.tile([1, BLK], F16)
        nc.vector.memset(ones_row[:], 1.0)
        pbb = ps_uy("ubprep")
        nc.tensor.matmul(pbb[0:BLK, 0, :], ones_row[:], bp16[:], start=True, stop=True)
        ub_bc = consts.tile([BLK, 2, C], F32)
        for b2 in range(2):
            nc.scalar.copy(ub_bc[:, b2, :], pbb[0:BLK, 0, :])

    # block-diag window mask [124, 496] fp16
    mask_np = np.zeros((BLK, GRP), dtype=np.float16)
    for j4 in range(4):
        for w in range(4):
            mask_np[w * WS:(w + 1) * WS, j4 * BLK + w * WS: j4 * BLK + (w + 1) * WS] = 1.0
    mask_d = nc.inline_tensor(mask_np, name="maskc")
    mask_sb = consts.tile([BLK, GRP], F16)
    nc.sync.dma_start(mask_sb[:], mask_d.ap())

    ones16 = consts.tile([BLK, 1], F16)
    nc.vector.memset(ones16[:], 16.0)

    # ---------------- software-pipelined loop (skew 3) ----------------
    # iter i: quads(i+1) | S0: z(i) | S1: scores/exp/mask(i-1)
    #         | S2: sums/recip/u(i-2) | S3: AV/y-scale/DMA(i-3)
    state = {}
    quads_done = 3

    def s0(g):
        t0 = g * GRP
        pz = ps_z(f"pz_{g}")
        for jt in range(2):
            for cs in range(2):
                nc.tensor.matmul(
                    pz[:, jt, 0:GRP], A16[:, cs, jt * 128:(jt + 1) * 128],
                    xT16[:, cs, t0:t0 + GRP], start=(cs == 0), stop=(cs == 1),
                )
        z16 = z_pool.tile([128, 2, GRP], F16, tag="z16", name=f"z16_{g}")
        if zb is None:
            nc.scalar.activation(z16[:], pz[:, :, 0:GRP], Identity, bias=0.0, scale=1.0)
        else:
            for jt in range(2):
                nc.scalar.activation(z16[:, jt, :], pz[:, jt, 0:GRP], Identity,
                                     bias=zb[:, jt:jt + 1], scale=1.0)
        state[("z16", g)] = z16

    def s1(g):
        t0 = g * GRP
        z16 = state.pop(("z16", g))
        psc = ps_s(f"psc_{g}")
        for b4 in range(4):
            for cs in range(2):
                nc.tensor.matmul(
                    psc[:, b4 * BLK:(b4 + 1) * BLK],
                    xT16[:, cs, t0 + b4 * BLK: t0 + b4 * BLK + 128],
                    z16[:, cs, b4 * BLK:(b4 + 1) * BLK],
                    start=(cs == 0), stop=(cs == 1),
                )
        attn = attn_pool.tile([BLK, GRP], F16, tag="at", name=f"at_{g}")
        nc.scalar.activation(attn[:], psc[0:BLK, 0:GRP], Exp, scale=1.0 / 256.0)
        nc.gpsimd.tensor_mul(attn[:], attn[:], mask_sb[:])
        state[("psc", g)] = psc
        state[("attn", g)] = attn

    def s2(g):
        t0 = g * GRP
        psc = state.pop(("psc", g))
        attn = state[("attn", g)]
        for b4 in range(4):
            nc.tensor.matmul(
                psc[0:BLK, GRP + b4:GRP + b4 + 1],
                attn[:, b4 * BLK:(b4 + 1) * BLK], ones16[:],
                start=True, stop=True,
            )
        recip = small_pool.tile([BLK, 4], F32, tag="rc", name=f"rc_{g}")
        nc.vector.reciprocal(recip[:], psc[0:BLK, GRP:GRP + 4])
        u16 = u_pool.tile([BLK, 4, C], F16, tag="u16", name=f"u16_{g}")
        for h in range(2):
            pu = ps_uy(f"pu_{g}_{h}")
            for b2 in range(2):
                b4 = 2 * h + b2
                for cs in range(2):
                    nc.tensor.matmul(
                        pu[0:BLK, b2, :],
                        xT16[:, cs, t0 + b4 * BLK: t0 + b4 * BLK + BLK],
                        Wu16[:, cs, :], start=(cs == 0), stop=(cs == 1),
                    )
            if ub_bc is not None:
                nc.vector.tensor_add(u16[:, 2 * h:2 * h + 2, :], pu[0:BLK, :, :],
                                     ub_bc[:])
            elif h == 1 and g % 2 == 1:
                nc.vector.tensor_copy(u16[:, 2 * h:2 * h + 2, :], pu[0:BLK, :, :])
            else:
                nc.scalar.copy(u16[:, 2 * h:2 * h + 2, :], pu[0:BLK, :, :])
        state[("recip", g)] = recip
        state[("u16", g)] = u16

    def s3(g):
        t0 = g * GRP
        attn = state.pop(("attn", g))
        recip = state.pop(("recip", g))
        u16 = state.pop(("u16", g))
        y_sb = y_pool.tile([BLK, 4, C], F32, tag="y", name=f"y_{g}")
        for h in range(2):
            py = ps_uy(f"py_{g}_{h}")
            for b2 in range(2):
                b4 = 2 * h + b2
                nc.tensor.matmul(
                    py[0:BLK, b2, :],
                    attn[:, b4 * BLK:(b4 + 1) * BLK], u16[:, b4, :],
                    start=True, stop=True,
                )
            for b2 in range(2):
                b4 = 2 * h + b2
                nc.vector.tensor_scalar(
                    y_sb[:, b4, :], py[0:BLK, b2, :], recip[:, b4:b4 + 1], None,
                    mybir.AluOpType.mult,
                )
            if g >= N_GRP - 2:
                nc.sync.dma_start(
                    y_d[t0 + h * 2 * BLK:t0 + (h + 1) * 2 * BLK, :]
                    .rearrange("(a p) c -> p a c", p=BLK),
                    y_sb[:, 2 * h:2 * h + 2, :],
                )
        if g < N_GRP - 2:
            nc.sync.dma_start(
                y_d[t0:t0 + GRP, :].rearrange("(a p) c -> p a c", p=BLK), y_sb[:]
            )

    for i in range(N_GRP + 3):
        if i - 3 >= 0:
            s3(i - 3)          # AV first: unblocks DVE y-scale early
        if i < N_GRP:
            # prefetch quads for group i+1 (group i's quads already resident)
            gq = min(i + 1, N_GRP - 1)
            need = min(N_QUAD, (gq * GRP + GRP + QUAD - 1) // QUAD)
            while quads_done < need:
                emit_quad(quads_done)
                quads_done += 1
            s0(i)
        if 0 <= i - 1 < N_GRP:
            s1(i - 1)
        if 0 <= i - 2 < N_GRP:
            s2(i - 2)

    ctx.close()


def kernel(x, Wqkv, bqkv, Wproj, bproj):
    from concourse.bass_utils import run_bass_kernel_spmd

    x = np.ascontiguousarray(np.asarray(x, dtype=np.float32))
    wqkv = np.ascontiguousarray(np.asarray(Wqkv, dtype=np.float32))
    bq = np.ascontiguousarray(np.asarray(bqkv, dtype=np.float32))
    wproj = np.ascontiguousarray(np.asarray(Wproj, dtype=np.float32))
    bp = np.ascontiguousarray(np.asarray(bproj, dtype=np.float32))

    key = (bool(np.any(bq)), bool(np.any(bp)))
    if _CACHE.get("key") != key:
        _CACHE["nc"] = _build_nc(*key)
        _CACHE["key"] = key
    nc = _CACHE["nc"]

    in_maps = [
        {"x": x[b], "wqkv": wqkv, "bqkv": bq, "wproj": wproj, "bproj": bp}
        for b in range(B)
    ]
    res = run_bass_kernel_spmd(nc, in_maps, core_ids=list(range(B)))
    return np.stack([r["y"] for r in res.results], axis=0)


# revision 11
# speedup vs baseline: 1.0904x; 1.0193x over previous
"""LocalAttention Trainium2 Bass kernel (folded weights, fp16 compute).

Problem: B=8, L=7936, C=256, WINDOW=31.  y = proj(attn(qkv(x))) with
window-local softmax (nW=256 windows of 31 tokens per batch element).

Sharding: data-parallel over batch; core b handles x[b] [7936, 256].

Algebraic folding (exact):
  scores*256 = x^T A16 x with A16 = (4Wq)^T (4Wk) = 16 Wq^T Wk applied as
    z = A16^T-contracted: z[c,q] = sum_cin A16'[cin,c] x[cin,q] with
    A16' = (4Wq)^T(4Wk) built so z = 16 Wk^T Wq x ... (verified vs reference)
  exp(psum/256); bq folds into a per-partition bias on z; the bk term is a
    per-query shift that softmax cancels exactly.
  y: Wu16 = 16 Wproj Wv; u = Wu16-applied + 16 bp; y = (sum_k e_k u_k)/(16 sum_k e_k)
  Sums use a 16-valued ones column so the 16s cancel in the reciprocal.

Per-core dataflow:
  - x quad [512 tok] f32 --SWDGE cast DMA--> fp16 staging -> PE transpose
    -> one PSUM bank -> DVE copy -> resident xT16 [c_lo, c_hi, t] fp16
  - z = A x: fp16 matmuls, 2 c-slices accumulated
  - scores per 124-token block: xT16 keys (lhsT) x z16 queries
  - exp on ACT (scale 1/256) -> attn fp16; window mask on GPSIMD
  - sums: 4 tiny PE matmuls into spare cols of the scores bank; DVE recip
  - u = Wu x: fp16, token-major, 124-aligned blocks
  - AV: y[t,c] = sum_k attn[k,t] u16[k,c]; y-scale: DVE tensor_scalar -> DMA

PSUM (8 banks): xt 1 (bufs=1) + z 2 (bufs=1) + s 2 (bufs=2, sums share bank)
  + uy 3 (shared u/y half-tile ring, bufs=3).
"""

import sys

for _p in ("/opt/trn_rl_repo",):
    if _p not in sys.path:
        sys.path.insert(0, _p)

import numpy as np

import concourse.bass as bass
import concourse.bacc as bacc_mod
import concourse.tile as tile
from concourse import mybir
from concourse.masks import make_identity

F32 = mybir.dt.float32
F16 = mybir.dt.float16
F8 = mybir.dt.float8e4
DR = mybir.MatmulPerfMode.DoubleRow
Identity = mybir.ActivationFunctionType.Identity
Exp = mybir.ActivationFunctionType.Exp

B, L, C = 8, 7936, 256
WS = 31
BLK = 4 * WS            # 124 tokens per attention block
GRP = 4 * BLK           # 496 tokens per group
N_GRP = L // GRP        # 16
QUAD = 512              # x staging granularity (tokens)
N_QUAD = (L + QUAD - 1) // QUAD   # 16 (last covers 256)
LP = L + 32             # padded xT16 width

_CACHE = {}


def _build_nc(has_bqkv, has_bproj):
    nc = bacc_mod.Bacc("TRN2", target_bir_lowering=False, debug=False, num_devices=8)
    x_d = nc.dram_tensor("x", [L, C], F32, kind="ExternalInput").ap()
    wqkv_d = nc.dram_tensor("wqkv", [3 * C, C], F32, kind="ExternalInput").ap()
    bqkv_d = nc.dram_tensor("bqkv", [3 * C], F32, kind="ExternalInput").ap()
    wproj_d = nc.dram_tensor("wproj", [C, C], F32, kind="ExternalInput").ap()
    bproj_d = nc.dram_tensor("bproj", [C], F32, kind="ExternalInput").ap()
    y_d = nc.dram_tensor("y", [L, C], F32, kind="ExternalOutput").ap()

    with tile.TileContext(nc) as tc:
        _emit(tc, x_d, wqkv_d, bqkv_d, wproj_d, bproj_d, y_d, has_bqkv, has_bproj)
    nc.compile()
    return nc


def _emit(tc, x_d, wqkv_d, bqkv_d, wproj_d, bproj_d, y_d, has_bqkv, has_bproj):
    nc = tc.nc
    from contextlib import ExitStack

    ctx = ExitStack()
    consts = ctx.enter_context(tc.tile_pool(name="consts", bufs=1))
    xt_pool = ctx.enter_context(tc.tile_pool(name="xt", bufs=1))
    xstage = ctx.enter_context(tc.tile_pool(name="xstage", bufs=3))
    z_pool = ctx.enter_context(tc.tile_pool(name="zp", bufs=3))
    attn_pool = ctx.enter_context(tc.tile_pool(name="ap", bufs=3))
    u_pool = ctx.enter_context(tc.tile_pool(name="up", bufs=3))
    y_pool = ctx.enter_context(tc.tile_pool(name="yp", bufs=3))
    small_pool = ctx.enter_context(tc.tile_pool(name="sp", bufs=3))

    # PSUM budget (8 banks): xt 1 + z 2 + s 2 + uy 3
    ps = ctx.enter_context(tc.tile_pool(name="ps", bufs=1, space="PSUM"))

    def ps_xt(name):
        return ps.tile([128, 4, 2, 128], F16, tag="xt", bufs=1, name=name)

    def ps_z(name):
        return ps.tile([128, 2, 512], F32, tag="z", bufs=1, name=name)

    def ps_s(name):
        return ps.tile([128, 512], F32, tag="s", bufs=2, name=name)

    def ps_uy(name):
        return ps.tile([128, 2, C], F32, tag="uy", bufs=3, name=name)

    # ---------------- constants / folded weights ----------------
    ident16 = consts.tile([128, 128], F16)
    make_identity(nc, ident16[:])

    # x staging: chunked SWDGE cast-DMAs (f32 -> fp16). First two chunks are
    # single quads for a fast pipeline start; later chunks amortize the
    # descriptor-generation cost on the Pool engine.
    xT16 = xt_pool.tile([128, 2, LP], F16)
    CHUNKS = [(0, 1), (1, 2), (2, 6), (6, 10), (10, 14), (14, 16)]
    _chunk_tiles = {}
    _quad_chunk = {}
    for ci, (q0, q1) in enumerate(CHUNKS):
        for q in range(q0, q1):
            _quad_chunk[q] = ci

    def start_chunk_dma(ci):
        q0, q1 = CHUNKS[ci]
        t0 = q0 * QUAD
        ntt = (min(q1 * QUAD, L) - t0) // 128
        xc = xstage.tile([128, 16, C], F16, tag="x16", name=f"x16_{ci}")
        nc.gpsimd.dma_start(
            xc[:, 0:ntt, :],
            x_d[t0:t0 + ntt * 128, :].rearrange("(a p) c -> p a c", p=128),
        )
        _chunk_tiles[ci] = xc

    def finish_quad(q):
        t0 = q * QUAD
        ntt = min(4, (L - t0) // 128)
        ci = _quad_chunk[q]
        xc = _chunk_tiles[ci]
        o = (q - CHUNKS[ci][0]) * 4
        pt = ps.tile([128, 4, 2, 128], F16, tag="xt", bufs=1, name=f"xtp_{q}")
        for tt in range(ntt):
            for cs in range(2):
                nc.tensor.transpose(
                    pt[:, tt, cs, :], xc[:, o + tt, cs * 128:(cs + 1) * 128],
                    ident16[:],
                )
        nc.vector.tensor_copy(
            xT16[:, :, t0:t0 + ntt * 128].rearrange("p c (t k) -> p c t k", t=ntt),
            pt[:, 0:ntt].rearrange("p t c k -> p c t k"),
        )

    _chunks_started = [0]

    def ensure_chunk(q):
        ci = _quad_chunk[q]
        while _chunks_started[0] <= ci:
            start_chunk_dma(_chunks_started[0])
            _chunks_started[0] += 1

    def emit_quad(q):
        ensure_chunk(min(q + 2, N_QUAD - 1))  # stay 2 quads ahead on DMAs
        finish_quad(q)

    start_chunk_dma(0)
    _chunks_started[0] = 1

    # Wqkv [768,256] -> j-major [128, 6, 256] fp16 (x4); o 0-1=q, 2-3=k, 4-5=v
    # (loaded via SWDGE so the first x chunk wins the DMA engines first)
    w_raw = consts.tile([128, 6, C], F32)
    nc.gpsimd.dma_start(w_raw[:], wqkv_d.rearrange("(o p) c -> p o c", p=128))
    ensure_chunk(2)

    wp_raw = consts.tile([128, 2, C], F32)
    nc.sync.dma_start(wp_raw[:], wproj_d.rearrange("(o p) c -> p o c", p=128))

    w16 = consts.tile([128, 6, C], F16)
    nc.scalar.activation(w16[:], w_raw[:], Identity, bias=0.0, scale=4.0)

    # A16 [cin_lo, cin_hi, cout] = (4Wq)^T (4Wk)  -- needs only w16
    A16 = consts.tile([128, 2, C], F16)
    Wu16 = consts.tile([128, 2, C], F16)
    for a in range(2):
        pa = ps_uy(f"wprep_a{a}")
        for js in range(2):
            nc.tensor.matmul(
                pa[:, 0, :], w16[:, js, a * 128:(a + 1) * 128], w16[:, 2 + js, :],
                start=(js == 0), stop=(js == 1),
            )
        nc.scalar.copy(A16[:, a, :], pa[:, 0, :])

    finish_quad(0)

    # Wproj -> jo-major fp16 (x4), transpose, then Wu16 = (4Wproj)(4Wv)
    wp16 = consts.tile([128, 2, C], F16)
    nc.scalar.activation(wp16[:], wp_raw[:], Identity, bias=0.0, scale=4.0)
    wp16T = consts.tile([128, 2, C], F16)
    ptw = ps.tile([128, 4, 2, 128], F16, tag="xt", bufs=1, name="ptw")
    for o in range(2):          # jo tile
        for ms in range(2):     # m slice
            nc.tensor.transpose(
                ptw[:, o, ms, :], wp16[:, o, ms * 128:(ms + 1) * 128], ident16[:]
            )
    nc.vector.tensor_copy(
        wp16T[:].rearrange("p m (o k) -> p o m k", o=2),
        ptw[:, 0:2],
    )
    for a in range(2):
        pw = ps_uy(f"wprep_u{a}")
        for ms in range(2):
            nc.tensor.matmul(
                pw[:, 0, :], w16[:, 4 + ms, a * 128:(a + 1) * 128], wp16T[:, ms, :],
                start=(ms == 0), stop=(ms == 1),
            )
        nc.scalar.copy(Wu16[:, a, :], pw[:, 0, :])

    for _q in range(1, 3):
        finish_quad(_q)

    # optional bias folds
    zb = None
    if has_bqkv:
        bq_sb = consts.tile([128, 6, 1], F32)
        nc.sync.dma_start(bq_sb[:], bqkv_d.rearrange("(o p) -> p o", p=128)[:, :, None])
        zb = consts.tile([128, 2], F32)
        for a in range(2):
            pb = ps_s(f"zb_{a}")
            for js in range(2):
                nc.tensor.matmul(
                    pb[:, 0:1], w_raw[:, 2 + js, a * 128:(a + 1) * 128],
                    bq_sb[:, js, :], start=(js == 0), stop=(js == 1),
                )
            nc.scalar.activation(zb[:, a:a + 1], pb[:, 0:1], Identity,
                                 bias=0.0, scale=16.0)
    ub_bc = None
    if has_bproj:
        bp_row = consts.tile([1, C], F32)
        nc.sync.dma_start(bp_row[:], bproj_d[None, :])
        bp16 = consts.tile([1, C], F16)
        nc.scalar.activation(bp16[:], bp_row[:], Identity, bias=0.0, scale=16.0)
        ones_row = consts.tile([1, BLK], F16)
        nc.vector.memset(ones_row[:], 1.0)
        pbb = ps_uy("ubprep")
        nc.tensor.matmul(pbb[0:BLK, 0, :], ones_row[:], bp16[:], start=True, stop=True)
        ub_bc = consts.tile([BLK, 2, C], F32)
        for b2 in range(2):
            nc.scalar.copy(ub_bc[:, b2, :], pbb[0:BLK, 0, :])

    # block-diag window mask [124, 496] fp16
    mask_np = np.zeros((BLK, GRP), dtype=np.float16)
    for j4 in range(4):
        for w in range(4):
            mask_np[w * WS:(w + 1) * WS, j4 * BLK + w * WS: j4 * BLK + (w + 1) * WS] = 1.0
    mask_d = nc.inline_tensor(mask_np, name="maskc")
    mask_sb = consts.tile([BLK, GRP], F16)
    nc.sync.dma_start(mask_sb[:], mask_d.ap())

    ones16 = consts.tile([BLK, 1], F16)
    nc.vector.memset(ones16[:], 16.0)

    # ---------------- software-pipelined loop (skew 3) ----------------
    # iter i: quads(i+1) | S0: z(i) | S1: scores/exp/mask(i-1)
    #         | S2: sums/recip/u(i-2) | S3: AV/y-scale/DMA(i-3)
    state = {}
    quads_done = 3

    def s0(g):
        t0 = g * GRP
        pz = ps_z(f"pz_{g}")
        for jt in range(2):
            for cs in range(2):
                nc.tensor.matmul(
                    pz[:, jt, 0:GRP], A16[:, cs, jt * 128:(jt + 1) * 128],
                    xT16[:, cs, t0:t0 + GRP], start=(cs == 0), stop=(cs == 1),
                )
        z16 = z_pool.tile([128, 2, GRP], F16, tag="z16", name=f"z16_{g}")
        if zb is None:
            nc.scalar.activation(z16[:], pz[:, :, 0:GRP], Identity, bias=0.0, scale=1.0)
        else:
            for jt in range(2):
                nc.scalar.activation(z16[:, jt, :], pz[:, jt, 0:GRP], Identity,
                                     bias=zb[:, jt:jt + 1], scale=1.0)
        state[("z16", g)] = z16

    def s1(g):
        t0 = g * GRP
        z16 = state.pop(("z16", g))
        psc = ps_s(f"psc_{g}")
        for b4 in range(4):
            for cs in range(2):
                nc.tensor.matmul(
                    psc[:, b4 * BLK:(b4 + 1) * BLK],
                    xT16[:, cs, t0 + b4 * BLK: t0 + b4 * BLK + 128],
                    z16[:, cs, b4 * BLK:(b4 + 1) * BLK],
                    start=(cs == 0), stop=(cs == 1),
                )
        attn = attn_pool.tile([BLK, GRP], F16, tag="at", name=f"at_{g}")
        nc.scalar.activation(attn[:], psc[0:BLK, 0:GRP], Exp, scale=1.0 / 256.0)
        if g >= N_GRP - 2:
            nc.vector.tensor_mul(attn[:], attn[:], mask_sb[:])
        else:
            nc.gpsimd.tensor_mul(attn[:], attn[:], mask_sb[:])
        state[("psc", g)] = psc
        state[("attn", g)] = attn

    def s2(g):
        t0 = g * GRP
        psc = state.pop(("psc", g))
        attn = state[("attn", g)]
        for b4 in range(4):
            nc.tensor.matmul(
                psc[0:BLK, GRP + b4:GRP + b4 + 1],
                attn[:, b4 * BLK:(b4 + 1) * BLK], ones16[:],
                start=True, stop=True,
            )
        recip = small_pool.tile([BLK, 4], F32, tag="rc", name=f"rc_{g}")
        nc.vector.reciprocal(recip[:], psc[0:BLK, GRP:GRP + 4])
        u16 = u_pool.tile([BLK, 4, C], F16, tag="u16", name=f"u16_{g}")
        for h in range(2):
            pu = ps_uy(f"pu_{g}_{h}")
            for b2 in range(2):
                b4 = 2 * h + b2
                for cs in range(2):
                    nc.tensor.matmul(
                        pu[0:BLK, b2, :],
                        xT16[:, cs, t0 + b4 * BLK: t0 + b4 * BLK + BLK],
                        Wu16[:, cs, :], start=(cs == 0), stop=(cs == 1),
                    )
            if ub_bc is not None:
                nc.vector.tensor_add(u16[:, 2 * h:2 * h + 2, :], pu[0:BLK, :, :],
                                     ub_bc[:])
            elif h == 1 and g % 2 == 1:
                nc.vector.tensor_copy(u16[:, 2 * h:2 * h + 2, :], pu[0:BLK, :, :])
            else:
                nc.scalar.copy(u16[:, 2 * h:2 * h + 2, :], pu[0:BLK, :, :])
        state[("recip", g)] = recip
        state[("u16", g)] = u16

    def s3(g):
        t0 = g * GRP
        attn = state.pop(("attn", g))
        recip = state.pop(("recip", g))
        u16 = state.pop(("u16", g))
        y_sb = y_pool.tile([BLK, 4, C], F32, tag="y", name=f"y_{g}")
        for h in range(2):
            py = ps_uy(f"py_{g}_{h}")
            for b2 in range(2):
                b4 = 2 * h + b2
                nc.tensor.matmul(
                    py[0:BLK, b2, :],
                    attn[:, b4 * BLK:(b4 + 1) * BLK], u16[:, b4, :],
                    start=True, stop=True,
                )
            for b2 in range(2):
                b4 = 2 * h + b2
                nc.vector.tensor_scalar(
                    y_sb[:, b4, :], py[0:BLK, b2, :], recip[:, b4:b4 + 1], None,
                    mybir.AluOpType.mult,
                )
            if g >= N_GRP - 2:
                nc.sync.dma_start(
                    y_d[t0 + h * 2 * BLK:t0 + (h + 1) * 2 * BLK, :]
                    .rearrange("(a p) c -> p a c", p=BLK),
                    y_sb[:, 2 * h:2 * h + 2, :],
                )
        if g < N_GRP - 2:
            nc.sync.dma_start(
                y_d[t0:t0 + GRP, :].rearrange("(a p) c -> p a c", p=BLK), y_sb[:]
            )

    for i in range(N_GRP + 3):
        if i - 3 >= 0:
            s3(i - 3)          # AV first: unblocks DVE y-scale early
        if i < N_GRP:
            # prefetch quads for group i+1 (group i's quads already resident)
            gq = min(i + 1, N_GRP - 1)
            need = min(N_QUAD, (gq * GRP + GRP + QUAD - 1) // QUAD)
            while quads_done < need:
                emit_quad(quads_done)
                quads_done += 1
            s0(i)
        if 0 <= i - 1 < N_GRP:
            s1(i - 1)
        if 0 <= i - 2 < N_GRP:
            s2(i - 2)

    ctx.close()


def kernel(x, Wqkv, bqkv, Wproj, bproj):
    from concourse.bass_utils import run_bass_kernel_spmd

    x = np.ascontiguousarray(np.asarray(x, dtype=np.float32))
    wqkv = np.ascontiguousarray(np.asarray(Wqkv, dtype=np.float32))
    bq = np.ascontiguousarray(np.asarray(bqkv, dtype=np.float32))
    wproj = np.ascontiguousarray(np.asarray(Wproj, dtype=np.float32))
    bp = np.ascontiguousarray(np.asarray(bproj, dtype=np.float32))

    key = (bool(np.any(bq)), bool(np.any(bp)))
    if _CACHE.get("key") != key:
        _CACHE["nc"] = _build_nc(*key)
        _CACHE["key"] = key
    nc = _CACHE["nc"]

    in_maps = [
        {"x": x[b], "wqkv": wqkv, "bqkv": bq, "wproj": wproj, "bproj": bp}
        for b in range(B)
    ]
    res = run_bass_kernel_spmd(nc, in_maps, core_ids=list(range(B)))
    return np.stack([r["y"] for r in res.results], axis=0)


# revision 12
# speedup vs baseline: 1.0909x; 1.0004x over previous
"""LocalAttention Trainium2 Bass kernel (folded weights, fp16 compute).

Problem: B=8, L=7936, C=256, WINDOW=31.  y = proj(attn(qkv(x))) with
window-local softmax (nW=256 windows of 31 tokens per batch element).

Sharding: data-parallel over batch; core b handles x[b] [7936, 256].

Algebraic folding (exact):
  scores*256 = x^T A16 x with A16 = (4Wq)^T (4Wk) = 16 Wq^T Wk applied as
    z = A16^T-contracted: z[c,q] = sum_cin A16'[cin,c] x[cin,q] with
    A16' = (4Wq)^T(4Wk) built so z = 16 Wk^T Wq x ... (verified vs reference)
  exp(psum/256); bq folds into a per-partition bias on z; the bk term is a
    per-query shift that softmax cancels exactly.
  y: Wu16 = 16 Wproj Wv; u = Wu16-applied + 16 bp; y = (sum_k e_k u_k)/(16 sum_k e_k)
  Sums use a 16-valued ones column so the 16s cancel in the reciprocal.

Per-core dataflow:
  - x quad [512 tok] f32 --SWDGE cast DMA--> fp16 staging -> PE transpose
    -> one PSUM bank -> DVE copy -> resident xT16 [c_lo, c_hi, t] fp16
  - z = A x: fp16 matmuls, 2 c-slices accumulated
  - scores per 124-token block: xT16 keys (lhsT) x z16 queries
  - exp on ACT (scale 1/256) -> attn fp16; window mask on GPSIMD
  - sums: 4 tiny PE matmuls into spare cols of the scores bank; DVE recip
  - u = Wu x: fp16, token-major, 124-aligned blocks
  - AV: y[t,c] = sum_k attn[k,t] u16[k,c]; y-scale: DVE tensor_scalar -> DMA

PSUM (8 banks): xt 1 (bufs=1) + z 2 (bufs=1) + s 2 (bufs=2, sums share bank)
  + uy 3 (shared u/y half-tile ring, bufs=3).
"""

import sys

for _p in ("/opt/trn_rl_repo",):
    if _p not in sys.path:
        sys.path.insert(0, _p)

import numpy as np

import concourse.bass as bass
import concourse.bacc as bacc_mod
import concourse.tile as tile
from concourse import mybir
from concourse.masks import make_identity

F32 = mybir.dt.float32
F16 = mybir.dt.float16
F8 = mybir.dt.float8e4
DR = mybir.MatmulPerfMode.DoubleRow
Identity = mybir.ActivationFunctionType.Identity
Exp = mybir.ActivationFunctionType.Exp

B, L, C = 8, 7936, 256
WS = 31
BLK = 4 * WS            # 124 tokens per attention block
GRP = 4 * BLK           # 496 tokens per group
N_GRP = L // GRP        # 16
QUAD = 512              # x staging granularity (tokens)
N_QUAD = (L + QUAD - 1) // QUAD   # 16 (last covers 256)
LP = L + 32             # padded xT16 width

_CACHE = {}


def _build_nc(has_bqkv, has_bproj):
    nc = bacc_mod.Bacc("TRN2", target_bir_lowering=False, debug=False, num_devices=8)
    x_d = nc.dram_tensor("x", [L, C], F32, kind="ExternalInput").ap()
    wqkv_d = nc.dram_tensor("wqkv", [3 * C, C], F32, kind="ExternalInput").ap()
    bqkv_d = nc.dram_tensor("bqkv", [3 * C], F32, kind="ExternalInput").ap()
    wproj_d = nc.dram_tensor("wproj", [C, C], F32, kind="ExternalInput").ap()
    bproj_d = nc.dram_tensor("bproj", [C], F32, kind="ExternalInput").ap()
    y_d = nc.dram_tensor("y", [L, C], F32, kind="ExternalOutput").ap()

    with tile.TileContext(nc) as tc:
        _emit(tc, x_d, wqkv_d, bqkv_d, wproj_d, bproj_d, y_d, has_bqkv, has_bproj)
    nc.compile()
    return nc


def _emit(tc, x_d, wqkv_d, bqkv_d, wproj_d, bproj_d, y_d, has_bqkv, has_bproj):
    nc = tc.nc
    from contextlib import ExitStack

    ctx = ExitStack()
    consts = ctx.enter_context(tc.tile_pool(name="consts", bufs=1))
    xt_pool = ctx.enter_context(tc.tile_pool(name="xt", bufs=1))
    xstage = ctx.enter_context(tc.tile_pool(name="xstage", bufs=3))
    z_pool = ctx.enter_context(tc.tile_pool(name="zp", bufs=3))
    attn_pool = ctx.enter_context(tc.tile_pool(name="ap", bufs=3))
    u_pool = ctx.enter_context(tc.tile_pool(name="up", bufs=3))
    y_pool = ctx.enter_context(tc.tile_pool(name="yp", bufs=3))
    small_pool = ctx.enter_context(tc.tile_pool(name="sp", bufs=3))

    # PSUM budget (8 banks): xt 1 + z 2 + s 2 + uy 3
    ps = ctx.enter_context(tc.tile_pool(name="ps", bufs=1, space="PSUM"))

    def ps_xt(name):
        return ps.tile([128, 4, 2, 128], F16, tag="xt", bufs=1, name=name)

    def ps_z(name):
        return ps.tile([128, 2, 512], F32, tag="z", bufs=1, name=name)

    def ps_s(name):
        return ps.tile([128, 512], F32, tag="s", bufs=2, name=name)

    def ps_uy(name):
        return ps.tile([128, 2, C], F32, tag="uy", bufs=3, name=name)

    # ---------------- constants / folded weights ----------------
    ident16 = consts.tile([128, 128], F16)
    make_identity(nc, ident16[:])

    # x staging: chunked SWDGE cast-DMAs (f32 -> fp16). First two chunks are
    # single quads for a fast pipeline start; later chunks amortize the
    # descriptor-generation cost on the Pool engine.
    xT16 = xt_pool.tile([128, 2, LP], F16)
    CHUNKS = [(0, 1), (1, 2), (2, 6), (6, 10), (10, 14), (14, 16)]
    _chunk_tiles = {}
    _quad_chunk = {}
    for ci, (q0, q1) in enumerate(CHUNKS):
        for q in range(q0, q1):
            _quad_chunk[q] = ci

    def start_chunk_dma(ci):
        q0, q1 = CHUNKS[ci]
        t0 = q0 * QUAD
        ntt = (min(q1 * QUAD, L) - t0) // 128
        xc = xstage.tile([128, 16, C], F16, tag="x16", name=f"x16_{ci}")
        nc.gpsimd.dma_start(
            xc[:, 0:ntt, :],
            x_d[t0:t0 + ntt * 128, :].rearrange("(a p) c -> p a c", p=128),
        )
        _chunk_tiles[ci] = xc

    def finish_quad(q):
        t0 = q * QUAD
        ntt = min(4, (L - t0) // 128)
        ci = _quad_chunk[q]
        xc = _chunk_tiles[ci]
        o = (q - CHUNKS[ci][0]) * 4
        pt = ps.tile([128, 4, 2, 128], F16, tag="xt", bufs=1, name=f"xtp_{q}")
        for tt in range(ntt):
            for cs in range(2):
                nc.tensor.transpose(
                    pt[:, tt, cs, :], xc[:, o + tt, cs * 128:(cs + 1) * 128],
                    ident16[:],
                )
        nc.vector.tensor_copy(
            xT16[:, :, t0:t0 + ntt * 128].rearrange("p c (t k) -> p c t k", t=ntt),
            pt[:, 0:ntt].rearrange("p t c k -> p c t k"),
        )

    _chunks_started = [0]

    def ensure_chunk(q):
        ci = _quad_chunk[q]
        while _chunks_started[0] <= ci:
            start_chunk_dma(_chunks_started[0])
            _chunks_started[0] += 1

    def emit_quad(q):
        ensure_chunk(min(q + 2, N_QUAD - 1))  # stay 2 quads ahead on DMAs
        finish_quad(q)

    start_chunk_dma(0)
    _chunks_started[0] = 1

    # Wqkv [768,256] -> j-major; qk half first (feeds A16), v half later.
    # (SWDGE so the first x chunk wins the DMA engines first)
    wqk_raw = consts.tile([128, 4, C], F32)
    nc.gpsimd.dma_start(wqk_raw[:], wqkv_d[0:512].rearrange("(o p) c -> p o c", p=128))
    ensure_chunk(2)
    wv_raw = consts.tile([128, 2, C], F32)
    nc.gpsimd.dma_start(wv_raw[:], wqkv_d[512:768].rearrange("(o p) c -> p o c", p=128))

    wp_raw = consts.tile([128, 2, C], F32)
    nc.sync.dma_start(wp_raw[:], wproj_d.rearrange("(o p) c -> p o c", p=128))

    w16 = consts.tile([128, 4, C], F16)
    nc.scalar.activation(w16[:], wqk_raw[:], Identity, bias=0.0, scale=4.0)
    w16v = consts.tile([128, 2, C], F16)
    nc.scalar.activation(w16v[:], wv_raw[:], Identity, bias=0.0, scale=4.0)

    # A16 [cin_lo, cin_hi, cout] = (4Wq)^T (4Wk)  -- needs only w16
    A16 = consts.tile([128, 2, C], F16)
    Wu16 = consts.tile([128, 2, C], F16)
    for a in range(2):
        pa = ps_uy(f"wprep_a{a}")
        for js in range(2):
            nc.tensor.matmul(
                pa[:, 0, :], w16[:, js, a * 128:(a + 1) * 128], w16[:, 2 + js, :],
                start=(js == 0), stop=(js == 1),
            )
        nc.scalar.copy(A16[:, a, :], pa[:, 0, :])

    finish_quad(0)

    # Wproj -> jo-major fp16 (x4), transpose, then Wu16 = (4Wproj)(4Wv)
    wp16 = consts.tile([128, 2, C], F16)
    nc.scalar.activation(wp16[:], wp_raw[:], Identity, bias=0.0, scale=4.0)
    wp16T = consts.tile([128, 2, C], F16)
    ptw = ps.tile([128, 4, 2, 128], F16, tag="xt", bufs=1, name="ptw")
    for o in range(2):          # jo tile
        for ms in range(2):     # m slice
            nc.tensor.transpose(
                ptw[:, o, ms, :], wp16[:, o, ms * 128:(ms + 1) * 128], ident16[:]
            )
    nc.vector.tensor_copy(
        wp16T[:].rearrange("p m (o k) -> p o m k", o=2),
        ptw[:, 0:2],
    )
    for a in range(2):
        pw = ps_uy(f"wprep_u{a}")
        for ms in range(2):
            nc.tensor.matmul(
                pw[:, 0, :], w16v[:, ms, a * 128:(a + 1) * 128], wp16T[:, ms, :],
                start=(ms == 0), stop=(ms == 1),
            )
        nc.scalar.copy(Wu16[:, a, :], pw[:, 0, :])

    for _q in range(1, 3):
        finish_quad(_q)

    # optional bias folds
    zb = None
    if has_bqkv:
        bq_sb = consts.tile([128, 6, 1], F32)
        nc.sync.dma_start(bq_sb[:], bqkv_d.rearrange("(o p) -> p o", p=128)[:, :, None])
        zb = consts.tile([128, 2], F32)
        for a in range(2):
            pb = ps_s(f"zb_{a}")
            for js in range(2):
                nc.tensor.matmul(
                    pb[:, 0:1], wqk_raw[:, 2 + js, a * 128:(a + 1) * 128],
                    bq_sb[:, js, :], start=(js == 0), stop=(js == 1),
                )
            nc.scalar.activation(zb[:, a:a + 1], pb[:, 0:1], Identity,
                                 bias=0.0, scale=16.0)
    ub_bc = None
    if has_bproj:
        bp_row = consts.tile([1, C], F32)
        nc.sync.dma_start(bp_row[:], bproj_d[None, :])
        bp16 = consts.tile([1, C], F16)
        nc.scalar.activation(bp16[:], bp_row[:], Identity, bias=0.0, scale=16.0)
        ones_row = consts.tile([1, BLK], F16)
        nc.vector.memset(ones_row[:], 1.0)
        pbb = ps_uy("ubprep")
        nc.tensor.matmul(pbb[0:BLK, 0, :], ones_row[:], bp16[:], start=True, stop=True)
        ub_bc = consts.tile([BLK, 2, C], F32)
        for b2 in range(2):
            nc.scalar.copy(ub_bc[:, b2, :], pbb[0:BLK, 0, :])

    # block-diag window mask [124, 496] fp16
    mask_np = np.zeros((BLK, GRP), dtype=np.float16)
    for j4 in range(4):
        for w in range(4):
            mask_np[w * WS:(w + 1) * WS, j4 * BLK + w * WS: j4 * BLK + (w + 1) * WS] = 1.0
    mask_d = nc.inline_tensor(mask_np, name="maskc")
    mask_sb = consts.tile([BLK, GRP], F16)
    nc.sync.dma_start(mask_sb[:], mask_d.ap())

    ones16 = consts.tile([BLK, 1], F16)
    nc.vector.memset(ones16[:], 16.0)

    # ---------------- software-pipelined loop (skew 3) ----------------
    # iter i: quads(i+1) | S0: z(i) | S1: scores/exp/mask(i-1)
    #         | S2: sums/recip/u(i-2) | S3: AV/y-scale/DMA(i-3)
    state = {}
    quads_done = 3

    def s0(g):
        t0 = g * GRP
        pz = ps_z(f"pz_{g}")
        for jt in range(2):
            for cs in range(2):
                nc.tensor.matmul(
                    pz[:, jt, 0:GRP], A16[:, cs, jt * 128:(jt + 1) * 128],
                    xT16[:, cs, t0:t0 + GRP], start=(cs == 0), stop=(cs == 1),
                )
        z16 = z_pool.tile([128, 2, GRP], F16, tag="z16", name=f"z16_{g}")
        if zb is None:
            nc.scalar.activation(z16[:], pz[:, :, 0:GRP], Identity, bias=0.0, scale=1.0)
        else:
            for jt in range(2):
                nc.scalar.activation(z16[:, jt, :], pz[:, jt, 0:GRP], Identity,
                                     bias=zb[:, jt:jt + 1], scale=1.0)
        state[("z16", g)] = z16

    def s1(g):
        t0 = g * GRP
        z16 = state.pop(("z16", g))
        psc = ps_s(f"psc_{g}")
        for b4 in range(4):
            for cs in range(2):
                nc.tensor.matmul(
                    psc[:, b4 * BLK:(b4 + 1) * BLK],
                    xT16[:, cs, t0 + b4 * BLK: t0 + b4 * BLK + 128],
                    z16[:, cs, b4 * BLK:(b4 + 1) * BLK],
                    start=(cs == 0), stop=(cs == 1),
                )
        attn = attn_pool.tile([BLK, GRP], F16, tag="at", name=f"at_{g}")
        nc.scalar.activation(attn[:], psc[0:BLK, 0:GRP], Exp, scale=1.0 / 256.0)
        if g >= N_GRP - 2:
            nc.vector.tensor_mul(attn[:], attn[:], mask_sb[:])
        else:
            nc.gpsimd.tensor_mul(attn[:], attn[:], mask_sb[:])
        state[("psc", g)] = psc
        state[("attn", g)] = attn

    def s2(g):
        t0 = g * GRP
        psc = state.pop(("psc", g))
        attn = state[("attn", g)]
        for b4 in range(4):
            nc.tensor.matmul(
                psc[0:BLK, GRP + b4:GRP + b4 + 1],
                attn[:, b4 * BLK:(b4 + 1) * BLK], ones16[:],
                start=True, stop=True,
            )
        recip = small_pool.tile([BLK, 4], F32, tag="rc", name=f"rc_{g}")
        nc.vector.reciprocal(recip[:], psc[0:BLK, GRP:GRP + 4])
        u16 = u_pool.tile([BLK, 4, C], F16, tag="u16", name=f"u16_{g}")
        for h in range(2):
            pu = ps_uy(f"pu_{g}_{h}")
            for b2 in range(2):
                b4 = 2 * h + b2
                for cs in range(2):
                    nc.tensor.matmul(
                        pu[0:BLK, b2, :],
                        xT16[:, cs, t0 + b4 * BLK: t0 + b4 * BLK + BLK],
                        Wu16[:, cs, :], start=(cs == 0), stop=(cs == 1),
                    )
            if ub_bc is not None:
                nc.vector.tensor_add(u16[:, 2 * h:2 * h + 2, :], pu[0:BLK, :, :],
                                     ub_bc[:])
            elif h == 1 and g % 2 == 1:
                nc.vector.tensor_copy(u16[:, 2 * h:2 * h + 2, :], pu[0:BLK, :, :])
            else:
                nc.scalar.copy(u16[:, 2 * h:2 * h + 2, :], pu[0:BLK, :, :])
        state[("recip", g)] = recip
        state[("u16", g)] = u16

    def s3(g):
        t0 = g * GRP
        attn = state.pop(("attn", g))
        recip = state.pop(("recip", g))
        u16 = state.pop(("u16", g))
        y_sb = y_pool.tile([BLK, 4, C], F32, tag="y", name=f"y_{g}")
        for h in range(2):
            py = ps_uy(f"py_{g}_{h}")
            for b2 in range(2):
                b4 = 2 * h + b2
                nc.tensor.matmul(
                    py[0:BLK, b2, :],
                    attn[:, b4 * BLK:(b4 + 1) * BLK], u16[:, b4, :],
                    start=True, stop=True,
                )
            for b2 in range(2):
                b4 = 2 * h + b2
                nc.vector.tensor_scalar(
                    y_sb[:, b4, :], py[0:BLK, b2, :], recip[:, b4:b4 + 1], None,
                    mybir.AluOpType.mult,
                )
            nc.sync.dma_start(
                y_d[t0 + h * 2 * BLK:t0 + (h + 1) * 2 * BLK, :]
                .rearrange("(a p) c -> p a c", p=BLK),
                y_sb[:, 2 * h:2 * h + 2, :],
            )

    for i in range(N_GRP + 3):
        if i - 3 >= 0:
            s3(i - 3)          # AV first: unblocks DVE y-scale early
        if i < N_GRP:
            # prefetch quads for group i+1 (group i's quads already resident)
            gq = min(i + 1, N_GRP - 1)
            need = min(N_QUAD, (gq * GRP + GRP + QUAD - 1) // QUAD)
            while quads_done < need:
                emit_quad(quads_done)
                quads_done += 1
            s0(i)
        if 0 <= i - 1 < N_GRP:
            s1(i - 1)
        if 0 <= i - 2 < N_GRP:
            s2(i - 2)

    ctx.close()


def kernel(x, Wqkv, bqkv, Wproj, bproj):
    from concourse.bass_utils import run_bass_kernel_spmd

    x = np.ascontiguousarray(np.asarray(x, dtype=np.float32))
    wqkv = np.ascontiguousarray(np.asarray(Wqkv, dtype=np.float32))
    bq = np.ascontiguousarray(np.asarray(bqkv, dtype=np.float32))
    wproj = np.ascontiguousarray(np.asarray(Wproj, dtype=np.float32))
    bp = np.ascontiguousarray(np.asarray(bproj, dtype=np.float32))

    key = (bool(np.any(bq)), bool(np.any(bp)))
    if _CACHE.get("key") != key:
        _CACHE["nc"] = _build_nc(*key)
        _CACHE["key"] = key
    nc = _CACHE["nc"]

    in_maps = [
        {"x": x[b], "wqkv": wqkv, "bqkv": bq, "wproj": wproj, "bproj": bp}
        for b in range(B)
    ]
    res = run_bass_kernel_spmd(nc, in_maps, core_ids=list(range(B)))
    return np.stack([r["y"] for r in res.results], axis=0)
